# revision 30
# baseline (speedup 1.0000x reference)
# Involution2d (K=7) Trainium2 kernel — 8-core SPMD, batch+spatial sharding.
#
# Sharding: 8 cores = (batch b in 0..3) x (H-half in 0..1). Each core computes
# a [128, 32, 64] output block. Per core, on device:
#   1. kernel generation: 1x1 conv (BN folded) -> ReLU -> 1x1 conv ->
#      kerm [49, 2048] fp16 per-pixel involution kernels (+ bias, W-edge mask).
#   2. involution: acc[c, p] = sum_o kerm[o, p] * xw[c, p + shift_o], fp16.
#      Per offset the kerm row is broadcast to 128 partitions by a one-hot
#      selector matmul on TensorE (fp16: 1 cycle/col) and either evicted to
#      fp16 SBUF by ScalarE or consumed from PSUM by a fused DVE STT; some
#      offsets use GPSIMD partition_broadcast instead. Products run on DVE at
#      the 2x packed-fp16 rate (an odd-phase copy of x keeps every shifted
#      operand 4-byte aligned). Accumulation is split between identity-matmul
#      PSUM accumulation on TensorE (terminal sinks, keeps the PE dense and
#      p-state high) and DVE fp16 adds.
import numpy as np

EPS = 1e-5
KK = 7
C = 128
H = 64
W = 64
B = 4
HH = 32            # rows per core
P = HH * W         # 2048 output pixels per core
NIN = 4 + 38 * W + 4   # 2440: 3-row halos + 4-elem guard pads each side
XOFF = 4 + 3 * W
GEN_CHUNK = 512
NOFF = KK * KK     # 49 offsets

# --- static engine assignment ------------------------------------------------
# product path per offset: 'A' = PE sel-matmul + Act evict + DVE mult,
# 'B' = Pool partition_broadcast (SBUF->SBUF) + DVE mult,
# 'D' = PE sel-matmul + DVE fused STT (GPSIMD cannot touch PSUM on TRN2).
# add path: 'E' = PE identity-matmul PSUM accumulate, 'V' = DVE fp16 add,
# 'F' = pairs pre-summed on DVE, then one PE identity-matmul per pair.
ACT_N, POOL_N, DVE_N = 25, 18, 6
PE_ADD_N, PAIR_ADD_N = 47, 0
PEND_DEPTH = 3


def _interleave(counts):
    # largest-remainder round robin over keys -> list of len sum(counts)
    total = sum(counts.values())
    acc = {k: 0.0 for k in counts}
    out = []
    for _ in range(total):
        for k in acc:
            acc[k] += counts[k] / total
        k = max(acc, key=lambda q: acc[q])
        acc[k] -= 1.0
        out.append(k)
    return out


PATHS = _interleave({"A": ACT_N, "B": POOL_N, "D": DVE_N})
ADDS = _interleave({"E": PE_ADD_N, "F": PAIR_ADD_N,
                    "V": NOFF - PE_ADD_N - PAIR_ADD_N})

# number of identity-matmul emissions (for the accumulation stop flag)
_N_ID_GROUPS = PE_ADD_N + PAIR_ADD_N // 2 + PAIR_ADD_N % 2
if ADDS[ADDS.index("V")] == "V":  # first V writes the accumulator directly
    pass

_STATE = {}


def _build():
    import concourse.tile as tile
    from concourse import bacc, mybir

    f16 = mybir.dt.float16
    f32 = mybir.dt.float32
    AT = mybir.AluOpType
    nc = bacc.Bacc("TRN2", target_bir_lowering=False, debug=False)

    xw_d = nc.dram_tensor("xw", [C, NIN], f16, kind="ExternalInput").ap()
    w1sT_d = nc.dram_tensor("w1sT", [C, 32], f16, kind="ExternalInput").ap()
    b1f_d = nc.dram_tensor("b1f", [32, 1], f32, kind="ExternalInput").ap()
    w2T_d = nc.dram_tensor("w2T", [32, NOFF], f16, kind="ExternalInput").ap()
    b2f_d = nc.dram_tensor("b2f", [NOFF, 1], f32, kind="ExternalInput").ap()
    mask_d = nc.dram_tensor("maskt", [NOFF, P], f16, kind="ExternalInput").ap()
    esel_d = nc.dram_tensor("esel", [NOFF, NOFF * C], f16, kind="ExternalInput").ap()
    id_d = nc.dram_tensor("id128", [C, C], f16, kind="ExternalInput").ap()
    out_d = nc.dram_tensor("out", [C, P], f16, kind="ExternalOutput").ap()

    first_V = ADDS.index("V")

    with tile.TileContext(nc) as tc, nc.allow_low_precision(
        reason="involution tolerance 2e-2; fp16 everywhere"
    ):
        with (
            tc.tile_pool(name="consts", bufs=1) as cpool,
            tc.tile_pool(name="work", bufs=1) as wpool,
            tc.tile_pool(name="ppool", bufs=1, space="PSUM") as ppool,
        ):
            # spread the big input DMAs across the three DGE-capable queues
            esel_sb = cpool.tile([NOFF, NOFF * C], f16, tag="esel")
            nc.scalar.dma_start(esel_sb[:], esel_d)
            x_sb = cpool.tile([C, NIN], f16, tag="x")
            nc.sync.dma_start(x_sb[:], xw_d)
            # odd-phase copy so every shifted slice has a 4-byte-aligned base
            # (keeps the DVE 2x packed fp16 mode eligible)
            xo_sb = cpool.tile([C, NIN], f16, tag="xo")
            nc.gpsimd.dma_start(xo_sb[:, 0:NIN - 1], xw_d[:, 1:NIN])
            w1sT = cpool.tile([C, 32], f16, tag="w1")
            nc.sync.dma_start(w1sT[:], w1sT_d)
            b1f = cpool.tile([32, 1], f32, tag="b1")
            nc.sync.dma_start(b1f[:], b1f_d)
            w2T = cpool.tile([32, NOFF], f16, tag="w2")
            nc.sync.dma_start(w2T[:], w2T_d)
            b2f = cpool.tile([NOFF, 1], f32, tag="b2")
            nc.sync.dma_start(b2f[:], b2f_d)
            mask_sb = cpool.tile([NOFF, P], f16, tag="mask")
            nc.scalar.dma_start(mask_sb[:], mask_d)
            id_sb = cpool.tile([C, C], f16, tag="id")
            nc.gpsimd.dma_start(id_sb[:], id_d)

            f_sb = cpool.tile([32, P], f16, tag="f")
            kerm = cpool.tile([NOFF, P], f16, tag="kerm")
            acc_dve = cpool.tile([C, P], f16, tag="accv")

            # ---- kernel generation ----
            # software-pipelined one stage deep: the PE emits f1(ci+1) before
            # k2(ci) so it never idles waiting for the Act relu of chunk ci
            def gen_stage2(ci):
                sl = slice(ci * GEN_CHUNK, (ci + 1) * GEN_CHUNK)
                k2 = ppool.tile([NOFF, GEN_CHUNK], f32, tag="bc", bufs=2,
                                name="k2")
                nc.tensor.matmul(k2[:], w2T[:], f_sb[:, sl], start=True, stop=True)
                # kerm = (k2 + b2) * mask  (fused; DVE since Pool can't read PSUM)
                nc.vector.scalar_tensor_tensor(
                    out=kerm[:, sl], in0=k2[:], scalar=b2f[:],
                    in1=mask_sb[:, sl],
                    op0=AT.add, op1=AT.mult,
                )

            gprev = None
            for ci in range(P // GEN_CHUNK):
                sl = slice(ci * GEN_CHUNK, (ci + 1) * GEN_CHUNK)
                xsl = slice(XOFF + ci * GEN_CHUNK, XOFF + (ci + 1) * GEN_CHUNK)
                f1 = ppool.tile([32, GEN_CHUNK], f32, tag="bc", bufs=2)
                nc.tensor.matmul(f1[:], w1sT[:], x_sb[:, xsl], start=True, stop=True)
                nc.scalar.activation(
                    f_sb[:, sl], f1[:], mybir.ActivationFunctionType.Relu,
                    bias=b1f[:],
                )
                if gprev is not None:
                    gen_stage2(gprev)
                gprev = ci
            gen_stage2(gprev)

            # ---- involution ----
            acc_psum = ppool.tile([C, P], f32, tag="acc", bufs=1)
            q_started = [False] * 4
            id_groups = [0]
            pend = []   # deferred adds: (o, prod_tile)
            fbuf = []   # 'F' products awaiting a partner

            def xs_ap(a, n):
                if a % 2 == 0:
                    return x_sb[:, a: a + n]
                return xo_sb[:, a - 1: a - 1 + n]

            def emit_id_add(tile_ap):
                id_groups[0] += 1
                stop = id_groups[0] == _N_ID_GROUPS
                for q in range(4):
                    qs = slice(q * 512, (q + 1) * 512)
                    nc.tensor.matmul(
                        acc_psum[:, qs], id_sb[:], tile_ap[:, qs],
                        start=not q_started[q], stop=stop,
                        skip_group_check=True,
                    )
                    q_started[q] = True

            def emit_add(o, prod):
                if ADDS[o] == "V":
                    if o != first_V:
                        nc.vector.tensor_add(acc_dve[:], acc_dve[:], prod[:])
                elif ADDS[o] == "F":
                    fbuf.append(prod)
                    if len(fbuf) == 2:
                        pa, pb = fbuf
                        del fbuf[:]
                        t = wpool.tile([C, P], f16, tag="pairs", bufs=3,
                                       name="pairsum")
                        nc.vector.tensor_add(t[:], pa[:], pb[:])
                        emit_id_add(t)
                else:
                    emit_id_add(prod)

            for o in range(NOFF):
                ip, jp = divmod(o, KK)
                A = W * ip + jp + 1
                path = PATHS[o]
                into_acc = ADDS[o] == "V" and o == first_V
                if into_acc:
                    prod = None
                    dst = acc_dve
                else:
                    prod = wpool.tile([C, P], f16, tag="prod", bufs=8)
                    dst = prod
                xs_full = xs_ap(A, P)
                if path == "B":
                    # partition_broadcast reads partition 0 only: stage the row
                    krow = wpool.tile([1, P], f16, tag="krow", bufs=4)
                    nc.sync.dma_start(krow[:], kerm[o:o + 1, :])
                    bch = wpool.tile([C, P], f16, tag="bch", bufs=4)
                    nc.gpsimd.partition_broadcast(bch[:], krow[:])
                    nc.vector.tensor_mul(dst[:], xs_full, bch[:])
                else:
                    bch = None
                    if path == "A":
                        bch = wpool.tile([C, P], f16, tag="bch", bufs=4)
                    for h2 in range(2):
                        hs = slice(h2 * 1024, (h2 + 1) * 1024)
                        bcp = ppool.tile([C, 1024], f32, tag="bc", bufs=2)
                        for nb in range(2):
                            ks = slice(h2 * 1024 + nb * 512,
                                       h2 * 1024 + (nb + 1) * 512)
                            nc.tensor.matmul(
                                bcp[:, nb * 512:(nb + 1) * 512],
                                esel_sb[:, o * C:(o + 1) * C],
                                kerm[:, ks], start=True, stop=True,
                            )
                        if path == "A":
                            nc.scalar.copy(bch[:, hs], bcp[:])
                        else:
                            nc.vector.scalar_tensor_tensor(
                                out=dst[:, hs], in0=bcp[:], scalar=1.0,
                                in1=xs_ap(A + h2 * 1024, 1024),
                                op0=AT.mult, op1=AT.mult,
                            )
                    if path == "A":
                        nc.vector.tensor_mul(dst[:], xs_full, bch[:])
                # deferred adds (two offsets behind) keep engines pipelined
                if len(pend) >= PEND_DEPTH:
                    emit_add(*pend.pop(0))
                if not into_acc:
                    pend.append((o, prod))
            while pend:
                emit_add(*pend.pop(0))
            if fbuf:  # odd leftover pair member
                emit_id_add(fbuf.pop())

            # ---- combine + store ---- (two pipelined halves shrink the tail)
            out_sb = wpool.tile([C, P], f16, tag="outsb")
            for hh in range(2):
                hs = slice(hh * 1024, (hh + 1) * 1024)
                nc.vector.tensor_add(out_sb[:, hs], acc_dve[:, hs],
                                     acc_psum[:, hs])
                nc.sync.dma_start(out_d[:, hs], out_sb[:, hs])

    nc.compile()
    return nc


def _get_nc():
    if "nc" not in _STATE:
        _STATE["nc"] = _build()
    return _STATE["nc"]


def _host_prep(x, w1, b1, bn_gamma, bn_beta, bn_mean, bn_var, w2, b2):
    x = np.asarray(x, dtype=np.float32)
    scale = np.asarray(bn_gamma) / np.sqrt(np.asarray(bn_var) + EPS)
    w1s = (np.asarray(w1) * scale[:, None]).astype(np.float32)
    b1f = (np.asarray(b1) * scale + np.asarray(bn_beta)
           - np.asarray(bn_mean) * scale).astype(np.float32)
    w1sT = np.ascontiguousarray(w1s.T).astype(np.float16)        # [128, 32]
    w2T = np.ascontiguousarray(np.asarray(w2, np.float32).T).astype(np.float16)
    b1fc = np.ascontiguousarray(b1f[:, None]).astype(np.float32)  # [32, 1]
    b2fc = np.ascontiguousarray(
        np.asarray(b2, np.float32)[:, None]).astype(np.float32)   # [49, 1]

    # W-edge mask: kerm[o, p] = 0 where w + dj leaves the row
    wcol = np.arange(P, dtype=np.int64) % W
    maskt = np.zeros((NOFF, P), dtype=np.float16)
    for ipp in range(KK):
        for jpp in range(KK):
            dj = jpp - 3
            maskt[ipp * KK + jpp] = ((wcol + dj >= 0) & (wcol + dj < W))
    maskt = np.ascontiguousarray(maskt)

    esel = np.zeros((NOFF, NOFF * C), dtype=np.float16)
    for o in range(NOFF):
        esel[o, o * C:(o + 1) * C] = 1.0
    id128 = np.eye(C, dtype=np.float16)

    in_maps = []
    for core in range(8):
        b, half = divmod(core, 2)
        h0 = HH * half
        xw = np.zeros((C, NIN), dtype=np.float16)
        lo = max(0, h0 - 3)
        hi = min(H, h0 + HH + 3)
        # rows [lo, hi) -> xw positions 4 + 64*(row - h0 + 3)
        src = x[b, :, lo:hi, :].reshape(C, -1)
        start = 4 + W * (lo - h0 + 3)
        xw[:, start:start + src.shape[1]] = src.astype(np.float16)
        in_maps.append({
            "xw": xw, "w1sT": w1sT, "b1f": b1fc, "w2T": w2T,
            "b2f": b2fc, "maskt": maskt, "esel": esel, "id128": id128,
        })
    return in_maps


def run(inputs: dict, trace: bool = False):
    from concourse.bass_utils import run_bass_kernel_spmd

    nc = _get_nc()
    in_maps = _host_prep(**inputs)
    res = run_bass_kernel_spmd(
        nc, in_maps, core_ids=list(range(8)), trace=trace,
    )
    out = np.zeros((B, C, H, W), dtype=np.float32)
    for core in range(8):
        b, half = divmod(core, 2)
        h0 = HH * half
        out[b, :, h0:h0 + HH, :] = (
            res.results[core]["out"].astype(np.float32).reshape(C, HH, W))
    return out, res


def kernel(**inputs) -> np.ndarray:
    out, _ = run(inputs, trace=False)
    return out
